# revision 22
# baseline (speedup 1.0000x reference)
"""MoE ragged FFN kernel for Trainium2 (8 NeuronCores, expert-parallel).

Strategy
--------
* Router (RMSNorm -> scaled projection -> softmax -> top-k -> renorm) is
  computed on host with jax-on-CPU using exactly the reference ops, so the
  discrete expert choices match the reference bit-for-bit.
* Expert-parallel sharding: core e owns expert e's weights. Tokens routed to
  expert e are gathered on host, padded to a common capacity C, and shipped
  pre-transposed so the device only runs dense matmuls.
* Device (per core): Y1^T = Wg^T @ X^T (contraction F), GLU
  act = gelu_tanh(gate) * lin computed pairwise on 128-row tiles,
  Y2^T = Wl^T @ act (contraction H). Tokens are always the matmul moving/free
  dimension; features live on partitions. fp16 matmul inputs (value ranges
  here are far from fp16 limits; 8x finer quantization than bf16 at the same
  1 cycle/row PE rate), fp32 PSUM accumulate.
* Default structure ("v2") stages activations through DRAM: phase 1 streams
  wg exactly once with x SBUF-resident (per-chunk x tiles, chunk-major DRAM
  layout so the first matmul starts ~10us in); phase 2 splits H in halves
  with half A's act SBUF space reserved from program start (its loads overlap
  phase 1, so the phase transition has no bubble) and half B's load hidden
  under half A's compute. Two bf16 partial outputs are summed on host.
* bf16 matmul inputs: fp16 data toggles the PE datapath harder and trips the
  chip power throttler (PE 2.4 -> 2.0 GHz); bf16 stays at 2.4 GHz and its
  error is well within budget.
* Host combines: out[token] += combine_weight * per_expert_scale[e] * y.

Measured (8 cores, G=4 S=2048 F=2048 H=4096 E=8 k=2): HW exec ~1.43 ms
(PE wall-to-wall at ~181 ns per 430-col matmul; ~1.40 ms structural floor),
global L2 relative error ~4.7e-3 vs the fp32 reference. Occasional runs
catch a chip power-state episode (+~8%, uncontrollable).
"""

import os

import numpy as np

P = 128
RMS_EPS = 1e-6

# Matmul input precision: "bf16" (default), "f16", or "f32r".
# bf16 is deliberately preferred over f16: f16's 11 mantissa bits toggle the
# PE datapath harder and trip the chip's periodic power throttler (HAM state
# 31, PE drops 2.4->2.0 GHz for ~160us windows); bf16 stays at 2.4 GHz and
# its ~4e-3 rel err is well within budget.
MOE_DTYPE = os.environ.get("MOE_DTYPE", "bf16")
# Token-block size (matmul moving free dim; >=256 keeps fp32r at 1 cyc/row).
TB = int(os.environ.get("MOE_TB", "512"))

_NEFF_CACHE: dict = {}


def _route_numpy(x, w_router, router_scale, top_k):
    """Fallback router in numpy (used only if jax-on-CPU is unavailable)."""
    G, S, F = x.shape
    B = G * S
    var = np.mean(np.square(x), axis=-1, keepdims=True, dtype=np.float32)
    ri = x / np.sqrt(var + RMS_EPS)
    ri = ri * np.float32(1.0 / np.sqrt(np.float32(F))) * router_scale
    logits = (ri.reshape(B, F) @ w_router).astype(np.float32)
    m = logits.max(axis=-1, keepdims=True)
    e = np.exp(logits - m)
    probs = e / e.sum(axis=-1, keepdims=True)
    choices = np.argsort(-logits, axis=-1, kind="stable")[:, :top_k]
    sel = np.take_along_axis(probs, choices, axis=-1)
    renorm = sel.sum(axis=-1, keepdims=True)
    renorm = np.where(renorm > 0.0, renorm, np.float32(1.0))
    combine = (sel / renorm).astype(np.float32)
    return choices.astype(np.int64), combine


def _route(x, w_router, router_scale, top_k):
    """Reference-exact router on CPU via jax. Returns (choices, combine) as
    numpy arrays of shape (B, k)."""
    try:
        import jax
        import jax.numpy as jnp

        cpu = jax.devices("cpu")[0]
    except Exception:
        return _route_numpy(np.asarray(x, dtype=np.float32),
                            np.asarray(w_router), np.asarray(router_scale),
                            top_k)
    G, S, F = x.shape
    E = w_router.shape[1]
    with jax.default_device(cpu):
        xj = jax.device_put(np.asarray(x), cpu)
        wj = jax.device_put(np.asarray(w_router), cpu)
        rj = jax.device_put(np.asarray(router_scale), cpu)
        var = jnp.mean(jnp.square(xj), axis=-1, keepdims=True)
        ri = xj * jax.lax.rsqrt(var + RMS_EPS)
        root_size = jax.lax.rsqrt(jnp.array(F, dtype=ri.dtype))
        ri = ri * root_size * rj.astype(ri.dtype)
        logits = jnp.einsum("gsd,de->gse", ri, wj).astype(jnp.float32)
        probs = jax.nn.softmax(logits, axis=-1)
        _, choices = jax.lax.approx_max_k(logits, k=top_k)
        indicator = jax.nn.one_hot(choices, E, dtype=probs.dtype).sum(axis=-2)
        renorm = jnp.sum(indicator * probs, axis=-1, keepdims=True)
        renorm = jnp.where(renorm > 0.0, renorm, 1.0)
        weights = probs / renorm
        combine = jnp.take_along_axis(weights, choices, axis=-1)
    B = G * S
    return (
        np.asarray(choices).reshape(B, top_k),
        np.asarray(combine).reshape(B, top_k).astype(np.float32),
    )


def _build_nc(C, F, H, dtype_name):
    """Build + compile the per-core FFN program (same program on all cores)."""
    import concourse.mybir as mybir
    import concourse.tile as tile
    from concourse import bacc

    KF = F // P          # k-subtiles for stage 1 (contraction F)
    KH = H // P          # k-subtiles for stage 2 (contraction H)
    MG = 2 * H // P      # wg column tiles, gate/lin interleaved per 128
    MO = F // P          # output row tiles
    f32 = mybir.dt.float32
    dt_in = _mm_dt(mybir, dtype_name)
    dt_mm = dt_in

    def mm(ap):
        return ap.bitcast(dt_mm) if ap.dtype != dt_mm else ap

    nc = bacc.Bacc(None, target_bir_lowering=False)
    xT = nc.dram_tensor("xT", [P, KF, C], dt_in, kind="ExternalInput")
    wg = nc.dram_tensor("wg", [P, MG, KF, P], dt_in, kind="ExternalInput")
    wl = nc.dram_tensor("wl", [P, MO, KH, P], dt_in, kind="ExternalInput")
    yT = nc.dram_tensor("yT", [MO, P, C], f32, kind="ExternalOutput")

    # Equal-size blocks (<= TB): avoids a tiny LDWEIGHTS-bound tail block.
    # Sizes are kept even (fp32r matmuls require an even moving free dim).
    assert C % 2 == 0
    nblk = -(-C // TB)
    half_base, half_extra = divmod(C // 2, nblk)
    blocks = []
    c0 = 0
    for b in range(nblk):
        tb = 2 * (half_base + (1 if b < half_extra else 0))
        blocks.append((c0, tb))
        c0 += tb
    assert c0 == C

    # f32r tiles are 2x the size of bf16 — shrink pools to fit SBUF.
    wbufs = 4 if dtype_name != "f32r" else 2
    abufs = 2 if dtype_name != "f32r" else 1
    with tile.TileContext(nc) as tc:
        with (
            tc.tile_pool(name="xp", bufs=2) as xp,
            tc.tile_pool(name="wgp", bufs=wbufs) as wgp,
            tc.tile_pool(name="wlp", bufs=wbufs) as wlp,
            tc.tile_pool(name="actp", bufs=abufs) as actp,
            tc.tile_pool(name="gp", bufs=3) as gp,
            tc.tile_pool(name="op", bufs=3) as op,
            tc.tile_pool(name="ps1", bufs=4, space="PSUM") as ps1,
            tc.tile_pool(name="ps2", bufs=3, space="PSUM") as ps2,
            tc.tile_pool(name="warm", bufs=1) as warmp,
            tc.tile_pool(name="warmps", bufs=1, space="PSUM") as warmpsp,
        ):
            # PE warm-up: ~5us of dummy matmuls while the first DMAs land,
            # so the HAM clock gate is at 8/8 when real matmuls start.
            wtile = warmp.tile([P, TB], mybir.dt.bfloat16)
            nc.vector.memset(wtile[:], 0.0)
            wps = warmpsp.tile([P, TB], f32)
            for _ in range(10):
                nc.tensor.matmul(wps[:], lhsT=wtile[:, :P], rhs=wtile[:],
                                 start=True, stop=True)
            for (c0, tb) in blocks:
                x_sb = xp.tile([P, KF, TB], dt_in, tag="x")
                # One DMA per k-subtile: spreads across queues and lets the
                # first matmuls start as soon as subtile 0 lands.
                for kx in range(KF):
                    nc.sync.dma_start(x_sb[:, kx, :tb], xT[:, kx, c0:c0 + tb])
                act_sb = actp.tile([P, KH, TB], dt_in, tag="act")
                for i in range(KH):
                    wgt_g = wgp.tile([P, KF, P], dt_in, tag="wg")
                    wgt_l = wgp.tile([P, KF, P], dt_in, tag="wg")
                    nc.sync.dma_start(wgt_g[:], wg[:, 2 * i])
                    nc.sync.dma_start(wgt_l[:], wg[:, 2 * i + 1])
                    pg = ps1.tile([P, TB], f32, tag="ps1")
                    pl = ps1.tile([P, TB], f32, tag="ps1")
                    for k in range(KF):
                        nc.tensor.matmul(
                            pg[:, :tb],
                            lhsT=mm(wgt_g[:, k, :]),
                            rhs=mm(x_sb[:, k, :tb]),
                            start=(k == 0), stop=(k == KF - 1),
                        )
                    for k in range(KF):
                        nc.tensor.matmul(
                            pl[:, :tb],
                            lhsT=mm(wgt_l[:, k, :]),
                            rhs=mm(x_sb[:, k, :tb]),
                            start=(k == 0), stop=(k == KF - 1),
                        )
                    gtmp = gp.tile([P, TB], f32, tag="g")
                    nc.scalar.activation(
                        gtmp[:, :tb], pg[:, :tb],
                        mybir.ActivationFunctionType.Gelu_apprx_tanh,
                    )
                    nc.vector.tensor_mul(
                        out=act_sb[:, i, :tb], in0=gtmp[:, :tb], in1=pl[:, :tb]
                    )
                for m in range(MO):
                    wlt = wlp.tile([P, KH, P], dt_in, tag="wl")
                    nc.sync.dma_start(wlt[:], wl[:, m])
                    p2 = ps2.tile([P, TB], f32, tag="ps2")
                    for k in range(KH):
                        nc.tensor.matmul(
                            p2[:, :tb],
                            lhsT=mm(wlt[:, k, :]),
                            rhs=mm(act_sb[:, k, :tb]),
                            start=(k == 0), stop=(k == KH - 1),
                        )
                    o_sb = op.tile([P, TB], f32, tag="o")
                    nc.vector.tensor_copy(out=o_sb[:, :tb], in_=p2[:, :tb])
                    nc.sync.dma_start(yT[m][:, c0:c0 + tb], o_sb[:, :tb])
    nc.compile()
    return nc


def _even_chunks(C, limit):
    """Split [0, C) into near-equal even-sized chunks of size <= limit."""
    assert C % 2 == 0
    nblk = -(-C // limit)
    half_base, half_extra = divmod(C // 2, nblk)
    out = []
    c0 = 0
    for b in range(nblk):
        tb = 2 * (half_base + (1 if b < half_extra else 0))
        out.append((c0, tb))
        c0 += tb
    assert c0 == C
    return out


def _build_nc_dram_act(C, F, H, dtype_name):
    """Variant that stages the GLU activations through DRAM.

    Phase 1 loops wg column-tiles outermost over ALL tokens (wg streamed
    exactly once, x SBUF-resident), writing act to a DRAM scratch tensor.
    Phase 2 loops token chunks, re-streaming only the smaller wl. Total DMA
    ~330MB fp32 instead of ~480MB, and phase 1 has large DMA slack.
    """
    import concourse.mybir as mybir
    import concourse.tile as tile
    from concourse import bacc

    KF = F // P
    KH = H // P
    MG = 2 * H // P
    MO = F // P
    f32 = mybir.dt.float32
    dt_in = _mm_dt(mybir, dtype_name)

    NQ = 4               # phase-2 contraction split: H quarters
    KQ = KH // NQ
    nc = bacc.Bacc(None, target_bir_lowering=False)
    xT = nc.dram_tensor("xT", [P, KF, C], dt_in, kind="ExternalInput")
    wg = nc.dram_tensor("wg", [P, MG, KF, P], dt_in, kind="ExternalInput")
    wl = nc.dram_tensor("wl", [P, MO, KH, P], dt_in, kind="ExternalInput")
    # NQ partial outputs (one per H-quarter), summed on host.
    yT = nc.dram_tensor("yT", [NQ, MO, P, C], f32, kind="ExternalOutput")

    chunks = _even_chunks(C, TB)

    with tile.TileContext(nc) as tc:
        with (
            tc.tile_pool(name="dram", bufs=1, space="DRAM") as dramp,
            tc.tile_pool(name="ps1", bufs=4, space="PSUM") as ps1,
            tc.tile_pool(name="ps2", bufs=3, space="PSUM") as ps2,
            tc.tile_pool(name="warm", bufs=1) as warmp,
            tc.tile_pool(name="warmps", bufs=1, space="PSUM") as warmpsp,
        ):
            actd = dramp.tile([KH, P, C], dt_in)
            wtile = warmp.tile([P, TB], mybir.dt.bfloat16)
            nc.vector.memset(wtile[:], 0.0)
            wps = warmpsp.tile([P, TB], f32)
            for _ in range(10):
                nc.tensor.matmul(wps[:], lhsT=wtile[:, :P], rhs=wtile[:],
                                 start=True, stop=True)
            with (
                tc.tile_pool(name="xp", bufs=1) as xp,
                tc.tile_pool(name="wgp", bufs=4) as wgp,
                tc.tile_pool(name="gp", bufs=3) as gp,
                tc.tile_pool(name="oa", bufs=4) as oap,
            ):
                x_sb = xp.tile([P, KF, C], dt_in)
                # Chunk-major loads so the first token chunk is ready early;
                # scalar HWDGE ring keeps x off the wg (sync) ring.
                for (c0, tb) in chunks:
                    for kx in range(KF):
                        nc.scalar.dma_start(x_sb[:, kx, c0:c0 + tb],
                                            xT[:, kx, c0:c0 + tb])
                # Pairs 0 and 1 interleave across chunks: the PE then consumes
                # x chunks at half rate, staying behind the inbound x DMA
                # instead of stalling on it.
                NI = min(2, KH)
                units = [(i, c) for c in range(len(chunks)) for i in range(NI)]
                units += [(i, c) for i in range(NI, KH)
                          for c in range(len(chunks))]
                pair_tiles = {}
                for (i, c) in units:
                    if i not in pair_tiles:
                        wgt_g = wgp.tile([P, KF, P], dt_in, tag="wg")
                        wgt_l = wgp.tile([P, KF, P], dt_in, tag="wg")
                        nc.sync.dma_start(wgt_g[:], wg[:, 2 * i])
                        nc.sync.dma_start(wgt_l[:], wg[:, 2 * i + 1])
                        pair_tiles[i] = (wgt_g, wgt_l)
                    wgt_g, wgt_l = pair_tiles[i]
                    for (c0, tb) in [chunks[c]]:
                        pg = ps1.tile([P, TB], f32, tag="ps1")
                        pl = ps1.tile([P, TB], f32, tag="ps1")
                        for k in range(KF):
                            nc.tensor.matmul(
                                pg[:, :tb], lhsT=wgt_g[:, k, :],
                                rhs=x_sb[:, k, c0:c0 + tb],
                                start=(k == 0), stop=(k == KF - 1),
                            )
                        for k in range(KF):
                            nc.tensor.matmul(
                                pl[:, :tb], lhsT=wgt_l[:, k, :],
                                rhs=x_sb[:, k, c0:c0 + tb],
                                start=(k == 0), stop=(k == KF - 1),
                            )
                        gtmp = gp.tile([P, TB], f32, tag="g")
                        nc.scalar.activation(
                            gtmp[:, :tb], pg[:, :tb],
                            mybir.ActivationFunctionType.Gelu_apprx_tanh,
                        )
                        oa = oap.tile([P, TB], dt_in, tag="oa")
                        nc.vector.tensor_mul(
                            out=oa[:, :tb], in0=gtmp[:, :tb], in1=pl[:, :tb]
                        )
                        nc.sync.dma_start(actd[i][:, c0:c0 + tb], oa[:, :tb])
            with (
                tc.tile_pool(name="acp", bufs=2) as acp,
                tc.tile_pool(name="wlp", bufs=4) as wlp,
                tc.tile_pool(name="op", bufs=3) as op,
            ):
                def load_quarter(q):
                    # Whole-token act quarter, double-buffered; scalar HWDGE
                    # ring keeps these off the wl/output (sync) ring.
                    t = acp.tile([P, KQ, C], dt_in, tag="a")
                    for ki in range(KQ):
                        nc.scalar.dma_start(t[:, ki, :],
                                            actd[q * KQ + ki][:, :])
                    return t

                a_cur = load_quarter(0)
                for q in range(NQ):
                    a_sb = a_cur
                    a_cur = load_quarter(q + 1) if q + 1 < NQ else None
                    for m in range(MO):
                        wlt = wlp.tile([P, KQ, P], dt_in, tag="wl")
                        nc.sync.dma_start(wlt[:], wl[:, m, q * KQ:(q + 1) * KQ])
                        for (c0, tb) in chunks:
                            p2 = ps2.tile([P, TB], f32, tag="ps2")
                            for k in range(KQ):
                                nc.tensor.matmul(
                                    p2[:, :tb], lhsT=wlt[:, k, :],
                                    rhs=a_sb[:, k, c0:c0 + tb],
                                    start=(k == 0), stop=(k == KQ - 1),
                                )
                            o_sb = op.tile([P, TB], f32, tag="o")
                            nc.vector.tensor_copy(out=o_sb[:, :tb],
                                                  in_=p2[:, :tb])
                            nc.sync.dma_start(yT[q][m][:, c0:c0 + tb],
                                              o_sb[:, :tb])
    nc.compile()
    return nc


def _even_chunks_uniform(C, limit):
    """Split [0, C) into nblk chunks: first nblk-1 of equal even size TBP,
    a (possibly smaller) even tail. Returns (chunks, TBP, nblk)."""
    assert C % 2 == 0
    nblk = -(-C // limit)
    TBP = 2 * (-(-C // (2 * nblk)))
    chunks = [(i * TBP, TBP) for i in range(nblk - 1)]
    chunks.append(((nblk - 1) * TBP, C - (nblk - 1) * TBP))
    assert sum(tb for _, tb in chunks) == C and all(tb % 2 == 0 for _, tb in chunks)
    return chunks, TBP, nblk


def _build_nc_v2(C, F, H, dtype_name):
    """dram_act variant tuned from trace analysis.

    Phase 1 (act production) is unchanged in shape: wg streamed once with x
    SBUF-resident, act staged to DRAM. Differences vs _build_nc_dram_act:

    * Phase 2 splits H in HALVES (not quarters). Half A's SBUF space is
      reserved from program start, so its act subtile loads are issued inline
      during phase 1 (as each actd[i] finishes) and the phase transition has
      no act-load bubble. Half B loads into the space phase-1 pools free,
      overlapped with half A's ~230us of compute.
    * Only 2 fp32 partial outputs (35MB instead of 70MB of y DMA).
    * The first wg pair is loaded in k-quarters so the first real matmul
      only waits for a [P,4,P] slice, shrinking the startup stall.
    """
    import concourse.mybir as mybir
    import concourse.tile as tile
    from concourse import bacc

    KF = F // P
    KH = H // P
    MG = 2 * H // P
    MO = F // P
    f32 = mybir.dt.float32
    dt_in = _mm_dt(mybir, dtype_name)

    NQ = 2
    KQ = KH // NQ
    chunks, TBP, NBLK = _even_chunks_uniform(C, TB)
    nc = bacc.Bacc(None, target_bir_lowering=False)
    # x is packed chunk-major on host: one contiguous DMA per token chunk
    # (13.7KB/partition descriptors instead of 860B), so the first matmul
    # starts ~10us in instead of waiting ~60us for all of x.
    xT = nc.dram_tensor("xT", [P, NBLK, KF, TBP], dt_in, kind="ExternalInput")
    wg = nc.dram_tensor("wg", [P, MG, KF, P], dt_in, kind="ExternalInput")
    wl = nc.dram_tensor("wl", [P, MO, KH, P], dt_in, kind="ExternalInput")
    # bf16 partial outputs (summed in fp32 on host): halves output DMA.
    yT = nc.dram_tensor("yT", [NQ, MO, P, C], dt_in, kind="ExternalOutput")

    # ~200KB/partition at C=2144 vs ~208 usable; shed prefetch depth if a
    # more imbalanced routing inflates C so compile can't overflow SBUF.
    wl_bufs = 4 if C <= 2208 else 2
    oa_bufs = 3 if C <= 2400 else 2
    with tile.TileContext(nc) as tc:
        with (
            tc.tile_pool(name="dram", bufs=1, space="DRAM") as dramp,
            tc.tile_pool(name="acA", bufs=1) as acAp,
            tc.tile_pool(name="wlp", bufs=wl_bufs) as wlp,
            tc.tile_pool(name="op", bufs=oa_bufs) as op,
            tc.tile_pool(name="ps1", bufs=5, space="PSUM") as ps1,
            tc.tile_pool(name="ps2", bufs=3, space="PSUM") as ps2,
            tc.tile_pool(name="warm", bufs=1) as warmp,
        ):
            actd = dramp.tile([KH, P, C], dt_in)
            actA = acAp.tile([P, KQ, C], dt_in)
            wtile = warmp.tile([P, TB], mybir.dt.bfloat16)
            nc.vector.memset(wtile[:], 0.0)
            # Warm-up accumulator borrows a ps2 bank (ps2 is idle until
            # phase 2), freeing an 8th bank for deeper ps1 pipelining.
            wps = ps2.tile([P, TB], f32, tag="ps2")
            for _ in range(12):
                nc.tensor.matmul(wps[:], lhsT=wtile[:, :P], rhs=wtile[:],
                                 start=True, stop=True)
            with (
                tc.tile_pool(name="xp", bufs=1) as xp,
                tc.tile_pool(name="wgp", bufs=4) as wgp,
                tc.tile_pool(name="gp", bufs=3) as gp,
                tc.tile_pool(name="oa", bufs=oa_bufs) as oap,
            ):
                # Per-chunk x tiles: a chunk's matmuls depend only on that
                # chunk's single DMA, not on the whole x load.
                x_tiles = []
                for c in range(NBLK):
                    x_c = xp.tile([P, KF, TBP], dt_in, tag=f"x{c}")
                    nc.scalar.dma_start(x_c[:], xT[:, c])
                    x_tiles.append(x_c)
                NI = min(2, KH)
                units = [(i, c) for c in range(len(chunks)) for i in range(NI)]
                units += [(i, c) for i in range(NI, KH)
                          for c in range(len(chunks))]
                # Track, per i, after which unit the last act chunk is written
                last_unit_of_i = {}
                for u, (i, c) in enumerate(units):
                    last_unit_of_i[i] = u
                pair_tiles = {}
                oa_tiles = {}
                for u, (i, c) in enumerate(units):
                    if i not in pair_tiles:
                        wgt_g = wgp.tile([P, KF, P], dt_in, tag="wg")
                        wgt_l = wgp.tile([P, KF, P], dt_in, tag="wg")
                        if i == 0:
                            # k-quarter loads: first matmul waits on 1/4 tile
                            kq = KF // 4
                            for j in range(4):
                                nc.sync.dma_start(
                                    wgt_g[:, j * kq:(j + 1) * kq, :],
                                    wg[:, 2 * i, j * kq:(j + 1) * kq])
                            for j in range(4):
                                nc.sync.dma_start(
                                    wgt_l[:, j * kq:(j + 1) * kq, :],
                                    wg[:, 2 * i + 1, j * kq:(j + 1) * kq])
                        else:
                            nc.sync.dma_start(wgt_g[:], wg[:, 2 * i])
                            nc.sync.dma_start(wgt_l[:], wg[:, 2 * i + 1])
                        pair_tiles[i] = (wgt_g, wgt_l)
                    wgt_g, wgt_l = pair_tiles[i]
                    for (c0, tb) in [chunks[c]]:
                        x_c = x_tiles[c]
                        pg = ps1.tile([P, TB], f32, tag="ps1")
                        pl = ps1.tile([P, TB], f32, tag="ps1")
                        for k in range(KF):
                            nc.tensor.matmul(
                                pg[:, :tb], lhsT=wgt_g[:, k, :],
                                rhs=x_c[:, k, :tb],
                                start=(k == 0), stop=(k == KF - 1),
                            )
                        for k in range(KF):
                            nc.tensor.matmul(
                                pl[:, :tb], lhsT=wgt_l[:, k, :],
                                rhs=x_c[:, k, :tb],
                                start=(k == 0), stop=(k == KF - 1),
                            )
                        gtmp = gp.tile([P, TB], dt_in, tag="g")
                        nc.scalar.activation(
                            gtmp[:, :tb], pg[:, :tb],
                            mybir.ActivationFunctionType.Gelu_apprx_tanh,
                        )
                        if i not in oa_tiles:
                            oa = oap.tile([P, C], dt_in, tag="oa", name="oa")
                            oa_tiles[i] = oa
                        nc.vector.tensor_mul(
                            out=oa_tiles[i][:, c0:c0 + tb],
                            in0=gtmp[:, :tb], in1=pl[:, :tb]
                        )
                    if u == last_unit_of_i[i]:
                        # One whole-C act write per i: 4.3KB/partition
                        # descriptors instead of 5x 860B.
                        nc.sync.dma_start(actd[i][:, :], oa_tiles.pop(i)[:, :])
                    # As soon as half-A subtile i is fully in DRAM, pull it
                    # back into the reserved SBUF half-A buffer (overlaps
                    # with the rest of phase 1).
                    if i < KQ and u == last_unit_of_i[i]:
                        nc.scalar.dma_start(actA[:, i, :], actd[i][:, :])
            with tc.tile_pool(name="acB", bufs=1) as acBp:
                actB = acBp.tile([P, KQ, C], dt_in)
                for ki in range(KQ):
                    nc.scalar.dma_start(actB[:, ki, :], actd[KQ + ki][:, :])
                for q, a_sb in ((0, actA), (1, actB)):
                    for m in range(MO):
                        wlt = wlp.tile([P, KQ, P], dt_in, tag="wl")
                        nc.sync.dma_start(wlt[:], wl[:, m, q * KQ:(q + 1) * KQ])
                        # Whole-C output tile: one y DMA per (q, m) with
                        # 4.3KB/partition descriptors instead of 5x 860B.
                        # The very last tile drains per-chunk instead, so the
                        # kernel doesn't end on a full 0.55MB write.
                        last_tile = (q == NQ - 1 and m == MO - 1)
                        if not last_tile:
                            o_sb = op.tile([P, C], dt_in, tag="o")
                        for (c0, tb) in chunks:
                            p2 = ps2.tile([P, TB], f32, tag="ps2")
                            for k in range(KQ):
                                nc.tensor.matmul(
                                    p2[:, :tb], lhsT=wlt[:, k, :],
                                    rhs=a_sb[:, k, c0:c0 + tb],
                                    start=(k == 0), stop=(k == KQ - 1),
                                )
                            if last_tile:
                                o_c = op.tile([P, TB], dt_in, tag="oc",
                                              name="o_c")
                                nc.vector.tensor_copy(out=o_c[:, :tb],
                                                      in_=p2[:, :tb])
                                nc.sync.dma_start(yT[q][m][:, c0:c0 + tb],
                                                  o_c[:, :tb])
                            else:
                                nc.vector.tensor_copy(
                                    out=o_sb[:, c0:c0 + tb], in_=p2[:, :tb])
                        if not last_tile:
                            nc.sync.dma_start(yT[q][m][:, :], o_sb[:, :])
    nc.compile()
    return nc


MOE_STRUCT = os.environ.get("MOE_STRUCT", "v2")


def _get_nc(C, F, H, dtype_name):
    key = (C, F, H, dtype_name, TB, MOE_STRUCT)
    if key not in _NEFF_CACHE:
        build = {"dram_act": _build_nc_dram_act, "v2": _build_nc_v2}.get(
            MOE_STRUCT, _build_nc)
        _NEFF_CACHE[key] = build(C, F, H, dtype_name)
    return _NEFF_CACHE[key]


def _mm_dt(mybir, dtype_name):
    return {
        "f32r": mybir.dt.float32r,
        "bf16": mybir.dt.bfloat16,
        "f16": mybir.dt.float16,
    }[dtype_name]


def _np_in_dtype():
    if MOE_DTYPE == "f32r":
        return np.float32
    if MOE_DTYPE == "f16":
        return np.float16
    import ml_dtypes

    return ml_dtypes.bfloat16


def run(x, w_router, w_gating, w_linear, per_expert_scale, router_scale, top_k,
        trace=False):
    from concourse.bass_utils import run_bass_kernel_spmd

    x = np.asarray(x)
    w_router = np.asarray(w_router)
    w_gating = np.asarray(w_gating)
    w_linear = np.asarray(w_linear)
    per_expert_scale = np.asarray(per_expert_scale)
    router_scale = np.asarray(router_scale)
    k = int(top_k)

    G, S, F = x.shape
    E = w_router.shape[1]
    H = w_linear.shape[1]
    B = G * S
    assert E == 8, "expert-parallel mapping assumes 8 experts on 8 cores"
    KF, KH, MO = F // P, H // P, F // P

    choices, combine = _route(x, w_router, router_scale, k)
    wcopy = combine * per_expert_scale.astype(np.float32)[choices]

    cf = choices.reshape(-1)
    tok_of_copy = np.repeat(np.arange(B), k)
    idx_per_e = [np.nonzero(cf == e)[0] for e in range(E)]
    counts = np.array([len(ix) for ix in idx_per_e])
    C = max(512, int(-(-counts.max() // 32)) * 32)

    nc = _get_nc(C, F, H, MOE_DTYPE)
    dt_in = _np_in_dtype()

    xf = x.reshape(B, F)
    if MOE_STRUCT == "v2":
        chunks, TBP, NBLK = _even_chunks_uniform(C, TB)
    in_maps = []
    toks_per_e = []
    for e in range(E):
        toks = tok_of_copy[idx_per_e[e]]
        toks_per_e.append(toks)
        n_e = len(toks)
        if MOE_STRUCT == "v2":
            # xT [P, NBLK, KF, TBP] chunk-major: xT[p, c, ko, j] =
            # x[toks[chunk_c_start + j], ko*P + p]
            xc = np.zeros((C, KF, P), dtype=dt_in)
            xc[:n_e] = xf[toks].astype(dt_in).reshape(n_e, KF, P)
            xT = np.zeros((P, NBLK, KF, TBP), dtype=dt_in)
            for c, (c0, tb) in enumerate(chunks):
                xT[:, c, :, :tb] = xc[c0:c0 + tb].transpose(2, 1, 0)
        else:
            # xT [P, KF, C]: xT[p, ko, c] = x[toks[c], ko*P + p]
            xT = np.zeros((P, KF, C), dtype=dt_in)
            xT[:, :, :n_e] = (
                xf[toks].astype(dt_in).reshape(n_e, KF, P).transpose(2, 1, 0)
            )
        # wg [P, MG, KF, P]: m=2i+c -> gate (c=0) / lin (c=1) rows 128i..128i+127
        wgq = w_gating[e].reshape(2, KH, P, KF, P)        # (c, i, col, ko, p)
        wgt = np.ascontiguousarray(
            wgq.transpose(4, 1, 0, 3, 2).reshape(P, 2 * KH, KF, P)
        ).astype(dt_in)
        # wl [P, MO, KH, P]: wl[p, m, kh, col] = w_linear[e][kh*P+p, m*P+col]
        wlq = w_linear[e].reshape(KH, P, MO, P)           # (kh, p, m, col)
        wlt = np.ascontiguousarray(wlq.transpose(1, 2, 0, 3)).astype(dt_in)
        in_maps.append({"xT": xT, "wg": wgt, "wl": wlt})

    res = run_bass_kernel_spmd(
        nc, in_maps, core_ids=list(range(E)), trace=trace,
        trace_cores=list(range(E)) if trace else None,
    )

    out = np.zeros((B, F), dtype=np.float32)
    for e in range(E):
        toks = toks_per_e[e]
        n_e = len(toks)
        if n_e == 0:
            continue
        yT = res.results[e]["yT"]                         # [MO, P, C] or [NQ, MO, P, C]
        if yT.ndim == 4:
            yT = yT.sum(axis=0, dtype=np.float32)
        y = yT.transpose(2, 0, 1).reshape(C, F)[:n_e]
        w = wcopy.reshape(-1)[idx_per_e[e]][:, None]
        out[toks] += w * y
    return out.reshape(G, S, F), res


def kernel(**inputs) -> np.ndarray:
    out, _ = run(**inputs)
    return out



# revision 24
# speedup vs baseline: 1.1965x; 1.1965x over previous
"""MoE ragged FFN kernel for Trainium2 (8 NeuronCores, expert-parallel).

Strategy
--------
* Router (RMSNorm -> scaled projection -> softmax -> top-k -> renorm) is
  computed on host with jax-on-CPU using exactly the reference ops, so the
  discrete expert choices match the reference bit-for-bit.
* Expert-parallel sharding: core e owns expert e's weights. Tokens routed to
  expert e are gathered on host, padded to a common capacity C, and shipped
  pre-transposed so the device only runs dense matmuls.
* Device (per core): Y1^T = Wg^T @ X^T (contraction F), GLU
  act = gelu_tanh(gate) * lin computed pairwise on 128-row tiles,
  Y2^T = Wl^T @ act (contraction H). Tokens are always the matmul moving/free
  dimension; features live on partitions. fp16 matmul inputs (value ranges
  here are far from fp16 limits; 8x finer quantization than bf16 at the same
  1 cycle/row PE rate), fp32 PSUM accumulate.
* Default structure ("v2") stages activations through DRAM: phase 1 streams
  wg exactly once with x SBUF-resident (per-chunk x tiles, chunk-major DRAM
  layout so the first matmul starts ~10us in); phase 2 splits H in halves
  with half A's act SBUF space reserved from program start (its loads overlap
  phase 1, so the phase transition has no bubble) and half B's load hidden
  under half A's compute. Two bf16 partial outputs are summed on host.
* bf16 matmul inputs: fp16 data toggles the PE datapath harder and trips the
  chip power throttler (PE 2.4 -> 2.0 GHz); bf16 stays at 2.4 GHz and its
  error is well within budget.
* Host combines: out[token] += combine_weight * per_expert_scale[e] * y.

Measured (8 cores, G=4 S=2048 F=2048 H=4096 E=8 k=2): HW exec ~1.43 ms
(PE wall-to-wall at ~181 ns per 430-col matmul; ~1.40 ms structural floor),
global L2 relative error ~4.7e-3 vs the fp32 reference. Occasional runs
catch a chip power-state episode (+~8%, uncontrollable).
"""

import os

import numpy as np

P = 128
RMS_EPS = 1e-6

# Matmul input precision: "bf16" (default), "f16", or "f32r".
# bf16 is deliberately preferred over f16: f16's 11 mantissa bits toggle the
# PE datapath harder and trip the chip's periodic power throttler (HAM state
# 31, PE drops 2.4->2.0 GHz for ~160us windows); bf16 stays at 2.4 GHz and
# its ~4e-3 rel err is well within budget.
MOE_DTYPE = os.environ.get("MOE_DTYPE", "bf16")
# Token-block size (matmul moving free dim; >=256 keeps fp32r at 1 cyc/row).
TB = int(os.environ.get("MOE_TB", "512"))

_NEFF_CACHE: dict = {}


def _route_numpy(x, w_router, router_scale, top_k):
    """Fallback router in numpy (used only if jax-on-CPU is unavailable)."""
    G, S, F = x.shape
    B = G * S
    var = np.mean(np.square(x), axis=-1, keepdims=True, dtype=np.float32)
    ri = x / np.sqrt(var + RMS_EPS)
    ri = ri * np.float32(1.0 / np.sqrt(np.float32(F))) * router_scale
    logits = (ri.reshape(B, F) @ w_router).astype(np.float32)
    m = logits.max(axis=-1, keepdims=True)
    e = np.exp(logits - m)
    probs = e / e.sum(axis=-1, keepdims=True)
    choices = np.argsort(-logits, axis=-1, kind="stable")[:, :top_k]
    sel = np.take_along_axis(probs, choices, axis=-1)
    renorm = sel.sum(axis=-1, keepdims=True)
    renorm = np.where(renorm > 0.0, renorm, np.float32(1.0))
    combine = (sel / renorm).astype(np.float32)
    return choices.astype(np.int64), combine


def _route(x, w_router, router_scale, top_k):
    """Reference-exact router on CPU via jax. Returns (choices, combine) as
    numpy arrays of shape (B, k)."""
    try:
        import jax
        import jax.numpy as jnp

        cpu = jax.devices("cpu")[0]
    except Exception:
        return _route_numpy(np.asarray(x, dtype=np.float32),
                            np.asarray(w_router), np.asarray(router_scale),
                            top_k)
    G, S, F = x.shape
    E = w_router.shape[1]
    with jax.default_device(cpu):
        xj = jax.device_put(np.asarray(x), cpu)
        wj = jax.device_put(np.asarray(w_router), cpu)
        rj = jax.device_put(np.asarray(router_scale), cpu)
        var = jnp.mean(jnp.square(xj), axis=-1, keepdims=True)
        ri = xj * jax.lax.rsqrt(var + RMS_EPS)
        root_size = jax.lax.rsqrt(jnp.array(F, dtype=ri.dtype))
        ri = ri * root_size * rj.astype(ri.dtype)
        logits = jnp.einsum("gsd,de->gse", ri, wj).astype(jnp.float32)
        probs = jax.nn.softmax(logits, axis=-1)
        _, choices = jax.lax.approx_max_k(logits, k=top_k)
        indicator = jax.nn.one_hot(choices, E, dtype=probs.dtype).sum(axis=-2)
        renorm = jnp.sum(indicator * probs, axis=-1, keepdims=True)
        renorm = jnp.where(renorm > 0.0, renorm, 1.0)
        weights = probs / renorm
        combine = jnp.take_along_axis(weights, choices, axis=-1)
    B = G * S
    return (
        np.asarray(choices).reshape(B, top_k),
        np.asarray(combine).reshape(B, top_k).astype(np.float32),
    )


def _build_nc(C, F, H, dtype_name):
    """Build + compile the per-core FFN program (same program on all cores)."""
    import concourse.mybir as mybir
    import concourse.tile as tile
    from concourse import bacc

    KF = F // P          # k-subtiles for stage 1 (contraction F)
    KH = H // P          # k-subtiles for stage 2 (contraction H)
    MG = 2 * H // P      # wg column tiles, gate/lin interleaved per 128
    MO = F // P          # output row tiles
    f32 = mybir.dt.float32
    dt_in = _mm_dt(mybir, dtype_name)
    dt_mm = dt_in

    def mm(ap):
        return ap.bitcast(dt_mm) if ap.dtype != dt_mm else ap

    nc = bacc.Bacc(None, target_bir_lowering=False)
    xT = nc.dram_tensor("xT", [P, KF, C], dt_in, kind="ExternalInput")
    wg = nc.dram_tensor("wg", [P, MG, KF, P], dt_in, kind="ExternalInput")
    wl = nc.dram_tensor("wl", [P, MO, KH, P], dt_in, kind="ExternalInput")
    yT = nc.dram_tensor("yT", [MO, P, C], f32, kind="ExternalOutput")

    # Equal-size blocks (<= TB): avoids a tiny LDWEIGHTS-bound tail block.
    # Sizes are kept even (fp32r matmuls require an even moving free dim).
    assert C % 2 == 0
    nblk = -(-C // TB)
    half_base, half_extra = divmod(C // 2, nblk)
    blocks = []
    c0 = 0
    for b in range(nblk):
        tb = 2 * (half_base + (1 if b < half_extra else 0))
        blocks.append((c0, tb))
        c0 += tb
    assert c0 == C

    # f32r tiles are 2x the size of bf16 — shrink pools to fit SBUF.
    wbufs = 4 if dtype_name != "f32r" else 2
    abufs = 2 if dtype_name != "f32r" else 1
    with tile.TileContext(nc) as tc:
        with (
            tc.tile_pool(name="xp", bufs=2) as xp,
            tc.tile_pool(name="wgp", bufs=wbufs) as wgp,
            tc.tile_pool(name="wlp", bufs=wbufs) as wlp,
            tc.tile_pool(name="actp", bufs=abufs) as actp,
            tc.tile_pool(name="gp", bufs=3) as gp,
            tc.tile_pool(name="op", bufs=3) as op,
            tc.tile_pool(name="ps1", bufs=4, space="PSUM") as ps1,
            tc.tile_pool(name="ps2", bufs=3, space="PSUM") as ps2,
            tc.tile_pool(name="warm", bufs=1) as warmp,
            tc.tile_pool(name="warmps", bufs=1, space="PSUM") as warmpsp,
        ):
            # PE warm-up: ~5us of dummy matmuls while the first DMAs land,
            # so the HAM clock gate is at 8/8 when real matmuls start.
            wtile = warmp.tile([P, TB], mybir.dt.bfloat16)
            nc.vector.memset(wtile[:], 0.0)
            wps = warmpsp.tile([P, TB], f32)
            for _ in range(10):
                nc.tensor.matmul(wps[:], lhsT=wtile[:, :P], rhs=wtile[:],
                                 start=True, stop=True)
            for (c0, tb) in blocks:
                x_sb = xp.tile([P, KF, TB], dt_in, tag="x")
                # One DMA per k-subtile: spreads across queues and lets the
                # first matmuls start as soon as subtile 0 lands.
                for kx in range(KF):
                    nc.sync.dma_start(x_sb[:, kx, :tb], xT[:, kx, c0:c0 + tb])
                act_sb = actp.tile([P, KH, TB], dt_in, tag="act")
                for i in range(KH):
                    wgt_g = wgp.tile([P, KF, P], dt_in, tag="wg")
                    wgt_l = wgp.tile([P, KF, P], dt_in, tag="wg")
                    nc.sync.dma_start(wgt_g[:], wg[:, 2 * i])
                    nc.sync.dma_start(wgt_l[:], wg[:, 2 * i + 1])
                    pg = ps1.tile([P, TB], f32, tag="ps1")
                    pl = ps1.tile([P, TB], f32, tag="ps1")
                    for k in range(KF):
                        nc.tensor.matmul(
                            pg[:, :tb],
                            lhsT=mm(wgt_g[:, k, :]),
                            rhs=mm(x_sb[:, k, :tb]),
                            start=(k == 0), stop=(k == KF - 1),
                        )
                    for k in range(KF):
                        nc.tensor.matmul(
                            pl[:, :tb],
                            lhsT=mm(wgt_l[:, k, :]),
                            rhs=mm(x_sb[:, k, :tb]),
                            start=(k == 0), stop=(k == KF - 1),
                        )
                    gtmp = gp.tile([P, TB], f32, tag="g")
                    nc.scalar.activation(
                        gtmp[:, :tb], pg[:, :tb],
                        mybir.ActivationFunctionType.Gelu_apprx_tanh,
                    )
                    nc.vector.tensor_mul(
                        out=act_sb[:, i, :tb], in0=gtmp[:, :tb], in1=pl[:, :tb]
                    )
                for m in range(MO):
                    wlt = wlp.tile([P, KH, P], dt_in, tag="wl")
                    nc.sync.dma_start(wlt[:], wl[:, m])
                    p2 = ps2.tile([P, TB], f32, tag="ps2")
                    for k in range(KH):
                        nc.tensor.matmul(
                            p2[:, :tb],
                            lhsT=mm(wlt[:, k, :]),
                            rhs=mm(act_sb[:, k, :tb]),
                            start=(k == 0), stop=(k == KH - 1),
                        )
                    o_sb = op.tile([P, TB], f32, tag="o")
                    nc.vector.tensor_copy(out=o_sb[:, :tb], in_=p2[:, :tb])
                    nc.sync.dma_start(yT[m][:, c0:c0 + tb], o_sb[:, :tb])
    nc.compile()
    return nc


def _even_chunks(C, limit):
    """Split [0, C) into near-equal even-sized chunks of size <= limit."""
    assert C % 2 == 0
    nblk = -(-C // limit)
    half_base, half_extra = divmod(C // 2, nblk)
    out = []
    c0 = 0
    for b in range(nblk):
        tb = 2 * (half_base + (1 if b < half_extra else 0))
        out.append((c0, tb))
        c0 += tb
    assert c0 == C
    return out


def _build_nc_dram_act(C, F, H, dtype_name):
    """Variant that stages the GLU activations through DRAM.

    Phase 1 loops wg column-tiles outermost over ALL tokens (wg streamed
    exactly once, x SBUF-resident), writing act to a DRAM scratch tensor.
    Phase 2 loops token chunks, re-streaming only the smaller wl. Total DMA
    ~330MB fp32 instead of ~480MB, and phase 1 has large DMA slack.
    """
    import concourse.mybir as mybir
    import concourse.tile as tile
    from concourse import bacc

    KF = F // P
    KH = H // P
    MG = 2 * H // P
    MO = F // P
    f32 = mybir.dt.float32
    dt_in = _mm_dt(mybir, dtype_name)

    NQ = 4               # phase-2 contraction split: H quarters
    KQ = KH // NQ
    nc = bacc.Bacc(None, target_bir_lowering=False)
    xT = nc.dram_tensor("xT", [P, KF, C], dt_in, kind="ExternalInput")
    wg = nc.dram_tensor("wg", [P, MG, KF, P], dt_in, kind="ExternalInput")
    wl = nc.dram_tensor("wl", [P, MO, KH, P], dt_in, kind="ExternalInput")
    # NQ partial outputs (one per H-quarter), summed on host.
    yT = nc.dram_tensor("yT", [NQ, MO, P, C], f32, kind="ExternalOutput")

    chunks = _even_chunks(C, TB)

    with tile.TileContext(nc) as tc:
        with (
            tc.tile_pool(name="dram", bufs=1, space="DRAM") as dramp,
            tc.tile_pool(name="ps1", bufs=4, space="PSUM") as ps1,
            tc.tile_pool(name="ps2", bufs=3, space="PSUM") as ps2,
            tc.tile_pool(name="warm", bufs=1) as warmp,
            tc.tile_pool(name="warmps", bufs=1, space="PSUM") as warmpsp,
        ):
            actd = dramp.tile([KH, P, C], dt_in)
            wtile = warmp.tile([P, TB], mybir.dt.bfloat16)
            nc.vector.memset(wtile[:], 0.0)
            wps = warmpsp.tile([P, TB], f32)
            for _ in range(10):
                nc.tensor.matmul(wps[:], lhsT=wtile[:, :P], rhs=wtile[:],
                                 start=True, stop=True)
            with (
                tc.tile_pool(name="xp", bufs=1) as xp,
                tc.tile_pool(name="wgp", bufs=4) as wgp,
                tc.tile_pool(name="gp", bufs=3) as gp,
                tc.tile_pool(name="oa", bufs=4) as oap,
            ):
                x_sb = xp.tile([P, KF, C], dt_in)
                # Chunk-major loads so the first token chunk is ready early;
                # scalar HWDGE ring keeps x off the wg (sync) ring.
                for (c0, tb) in chunks:
                    for kx in range(KF):
                        nc.scalar.dma_start(x_sb[:, kx, c0:c0 + tb],
                                            xT[:, kx, c0:c0 + tb])
                # Pairs 0 and 1 interleave across chunks: the PE then consumes
                # x chunks at half rate, staying behind the inbound x DMA
                # instead of stalling on it.
                NI = min(2, KH)
                units = [(i, c) for c in range(len(chunks)) for i in range(NI)]
                units += [(i, c) for i in range(NI, KH)
                          for c in range(len(chunks))]
                pair_tiles = {}
                for (i, c) in units:
                    if i not in pair_tiles:
                        wgt_g = wgp.tile([P, KF, P], dt_in, tag="wg")
                        wgt_l = wgp.tile([P, KF, P], dt_in, tag="wg")
                        nc.sync.dma_start(wgt_g[:], wg[:, 2 * i])
                        nc.sync.dma_start(wgt_l[:], wg[:, 2 * i + 1])
                        pair_tiles[i] = (wgt_g, wgt_l)
                    wgt_g, wgt_l = pair_tiles[i]
                    for (c0, tb) in [chunks[c]]:
                        pg = ps1.tile([P, TB], f32, tag="ps1")
                        pl = ps1.tile([P, TB], f32, tag="ps1")
                        for k in range(KF):
                            nc.tensor.matmul(
                                pg[:, :tb], lhsT=wgt_g[:, k, :],
                                rhs=x_sb[:, k, c0:c0 + tb],
                                start=(k == 0), stop=(k == KF - 1),
                            )
                        for k in range(KF):
                            nc.tensor.matmul(
                                pl[:, :tb], lhsT=wgt_l[:, k, :],
                                rhs=x_sb[:, k, c0:c0 + tb],
                                start=(k == 0), stop=(k == KF - 1),
                            )
                        gtmp = gp.tile([P, TB], f32, tag="g")
                        nc.scalar.activation(
                            gtmp[:, :tb], pg[:, :tb],
                            mybir.ActivationFunctionType.Gelu_apprx_tanh,
                        )
                        oa = oap.tile([P, TB], dt_in, tag="oa")
                        nc.vector.tensor_mul(
                            out=oa[:, :tb], in0=gtmp[:, :tb], in1=pl[:, :tb]
                        )
                        nc.sync.dma_start(actd[i][:, c0:c0 + tb], oa[:, :tb])
            with (
                tc.tile_pool(name="acp", bufs=2) as acp,
                tc.tile_pool(name="wlp", bufs=4) as wlp,
                tc.tile_pool(name="op", bufs=3) as op,
            ):
                def load_quarter(q):
                    # Whole-token act quarter, double-buffered; scalar HWDGE
                    # ring keeps these off the wl/output (sync) ring.
                    t = acp.tile([P, KQ, C], dt_in, tag="a")
                    for ki in range(KQ):
                        nc.scalar.dma_start(t[:, ki, :],
                                            actd[q * KQ + ki][:, :])
                    return t

                a_cur = load_quarter(0)
                for q in range(NQ):
                    a_sb = a_cur
                    a_cur = load_quarter(q + 1) if q + 1 < NQ else None
                    for m in range(MO):
                        wlt = wlp.tile([P, KQ, P], dt_in, tag="wl")
                        nc.sync.dma_start(wlt[:], wl[:, m, q * KQ:(q + 1) * KQ])
                        for (c0, tb) in chunks:
                            p2 = ps2.tile([P, TB], f32, tag="ps2")
                            for k in range(KQ):
                                nc.tensor.matmul(
                                    p2[:, :tb], lhsT=wlt[:, k, :],
                                    rhs=a_sb[:, k, c0:c0 + tb],
                                    start=(k == 0), stop=(k == KQ - 1),
                                )
                            o_sb = op.tile([P, TB], f32, tag="o")
                            nc.vector.tensor_copy(out=o_sb[:, :tb],
                                                  in_=p2[:, :tb])
                            nc.sync.dma_start(yT[q][m][:, c0:c0 + tb],
                                              o_sb[:, :tb])
    nc.compile()
    return nc


def _even_chunks_uniform(C, limit):
    """Split [0, C) into nblk chunks: first nblk-1 of equal even size TBP,
    a (possibly smaller) even tail. Returns (chunks, TBP, nblk)."""
    assert C % 2 == 0
    nblk = -(-C // limit)
    TBP = 2 * (-(-C // (2 * nblk)))
    chunks = [(i * TBP, TBP) for i in range(nblk - 1)]
    chunks.append(((nblk - 1) * TBP, C - (nblk - 1) * TBP))
    assert sum(tb for _, tb in chunks) == C and all(tb % 2 == 0 for _, tb in chunks)
    return chunks, TBP, nblk


def _build_nc_v2(C, F, H, dtype_name):
    """dram_act variant tuned from trace analysis.

    Phase 1 (act production) is unchanged in shape: wg streamed once with x
    SBUF-resident, act staged to DRAM. Differences vs _build_nc_dram_act:

    * Phase 2 splits H in HALVES (not quarters). Half A's SBUF space is
      reserved from program start, so its act subtile loads are issued inline
      during phase 1 (as each actd[i] finishes) and the phase transition has
      no act-load bubble. Half B loads into the space phase-1 pools free,
      overlapped with half A's ~230us of compute.
    * Only 2 fp32 partial outputs (35MB instead of 70MB of y DMA).
    * The first wg pair is loaded in k-quarters so the first real matmul
      only waits for a [P,4,P] slice, shrinking the startup stall.
    """
    import concourse.mybir as mybir
    import concourse.tile as tile
    from concourse import bacc

    KF = F // P
    KH = H // P
    MG = 2 * H // P
    MO = F // P
    f32 = mybir.dt.float32
    dt_in = _mm_dt(mybir, dtype_name)

    NQ = 2
    KQ = KH // NQ
    chunks, TBP, NBLK = _even_chunks_uniform(C, TB)
    nc = bacc.Bacc(None, target_bir_lowering=False)
    # x is packed chunk-major on host: one contiguous DMA per token chunk
    # (13.7KB/partition descriptors instead of 860B), so the first matmul
    # starts ~10us in instead of waiting ~60us for all of x.
    xT = nc.dram_tensor("xT", [P, NBLK, KF, TBP], dt_in, kind="ExternalInput")
    wg = nc.dram_tensor("wg", [P, MG, KF, P], dt_in, kind="ExternalInput")
    wl = nc.dram_tensor("wl", [P, MO, KH, P], dt_in, kind="ExternalInput")
    # bf16 partial outputs (summed in fp32 on host): halves output DMA.
    yT = nc.dram_tensor("yT", [NQ, MO, P, C], dt_in, kind="ExternalOutput")

    # ~200KB/partition at C=2144 vs ~208 usable; shed prefetch depth if a
    # more imbalanced routing inflates C so compile can't overflow SBUF.
    wl_bufs = 4 if C <= 2208 else 2
    oa_bufs = 3 if C <= 2400 else 2
    with tile.TileContext(nc) as tc:
        with (
            tc.tile_pool(name="dram", bufs=1, space="DRAM") as dramp,
            tc.tile_pool(name="acA", bufs=1) as acAp,
            tc.tile_pool(name="wlp", bufs=wl_bufs) as wlp,
            tc.tile_pool(name="op", bufs=oa_bufs) as op,
            tc.tile_pool(name="ps1", bufs=4, space="PSUM") as ps1,
            tc.tile_pool(name="ps2", bufs=3, space="PSUM") as ps2,
            tc.tile_pool(name="warm", bufs=1) as warmp,
            tc.tile_pool(name="warmps", bufs=1, space="PSUM") as warmpsp,
        ):
            actd = dramp.tile([KH, P, C], dt_in)
            actA = acAp.tile([P, KQ, C], dt_in)
            wtile = warmp.tile([P, TB], mybir.dt.bfloat16)
            nc.vector.memset(wtile[:], 0.0)
            wps = warmpsp.tile([P, TB], f32)
            for _ in range(12):
                nc.tensor.matmul(wps[:], lhsT=wtile[:, :P], rhs=wtile[:],
                                 start=True, stop=True)
            with (
                tc.tile_pool(name="xp", bufs=1) as xp,
                tc.tile_pool(name="wgp", bufs=4) as wgp,
                tc.tile_pool(name="gp", bufs=3) as gp,
                tc.tile_pool(name="oa", bufs=oa_bufs) as oap,
            ):
                # Per-chunk x tiles: a chunk's matmuls depend only on that
                # chunk's single DMA, not on the whole x load.
                x_tiles = []
                for c in range(NBLK):
                    x_c = xp.tile([P, KF, TBP], dt_in, tag=f"x{c}")
                    nc.scalar.dma_start(x_c[:], xT[:, c])
                    x_tiles.append(x_c)
                NI = min(2, KH)
                units = [(i, c) for c in range(len(chunks)) for i in range(NI)]
                units += [(i, c) for i in range(NI, KH)
                          for c in range(len(chunks))]
                # Track, per i, after which unit the last act chunk is written
                last_unit_of_i = {}
                for u, (i, c) in enumerate(units):
                    last_unit_of_i[i] = u
                pair_tiles = {}
                oa_tiles = {}
                for u, (i, c) in enumerate(units):
                    if i not in pair_tiles:
                        wgt_g = wgp.tile([P, KF, P], dt_in, tag="wg")
                        wgt_l = wgp.tile([P, KF, P], dt_in, tag="wg")
                        if i == 0:
                            # k-quarter loads: first matmul waits on 1/4 tile
                            kq = KF // 4
                            for j in range(4):
                                nc.sync.dma_start(
                                    wgt_g[:, j * kq:(j + 1) * kq, :],
                                    wg[:, 2 * i, j * kq:(j + 1) * kq])
                            for j in range(4):
                                nc.sync.dma_start(
                                    wgt_l[:, j * kq:(j + 1) * kq, :],
                                    wg[:, 2 * i + 1, j * kq:(j + 1) * kq])
                        else:
                            nc.sync.dma_start(wgt_g[:], wg[:, 2 * i])
                            nc.sync.dma_start(wgt_l[:], wg[:, 2 * i + 1])
                        pair_tiles[i] = (wgt_g, wgt_l)
                    wgt_g, wgt_l = pair_tiles[i]
                    for (c0, tb) in [chunks[c]]:
                        x_c = x_tiles[c]
                        pg = ps1.tile([P, TB], f32, tag="ps1")
                        pl = ps1.tile([P, TB], f32, tag="ps1")
                        for k in range(KF):
                            nc.tensor.matmul(
                                pg[:, :tb], lhsT=wgt_g[:, k, :],
                                rhs=x_c[:, k, :tb],
                                start=(k == 0), stop=(k == KF - 1),
                            )
                        for k in range(KF):
                            nc.tensor.matmul(
                                pl[:, :tb], lhsT=wgt_l[:, k, :],
                                rhs=x_c[:, k, :tb],
                                start=(k == 0), stop=(k == KF - 1),
                            )
                        gtmp = gp.tile([P, TB], dt_in, tag="g")
                        nc.scalar.activation(
                            gtmp[:, :tb], pg[:, :tb],
                            mybir.ActivationFunctionType.Gelu_apprx_tanh,
                        )
                        if i not in oa_tiles:
                            oa = oap.tile([P, C], dt_in, tag="oa", name="oa")
                            oa_tiles[i] = oa
                        nc.vector.tensor_mul(
                            out=oa_tiles[i][:, c0:c0 + tb],
                            in0=gtmp[:, :tb], in1=pl[:, :tb]
                        )
                    if u == last_unit_of_i[i]:
                        # One whole-C act write per i: 4.3KB/partition
                        # descriptors instead of 5x 860B.
                        nc.sync.dma_start(actd[i][:, :], oa_tiles.pop(i)[:, :])
                    # As soon as half-A subtile i is fully in DRAM, pull it
                    # back into the reserved SBUF half-A buffer (overlaps
                    # with the rest of phase 1).
                    if i < KQ and u == last_unit_of_i[i]:
                        nc.scalar.dma_start(actA[:, i, :], actd[i][:, :])
            with tc.tile_pool(name="acB", bufs=1) as acBp:
                actB = acBp.tile([P, KQ, C], dt_in)
                for ki in range(KQ):
                    nc.scalar.dma_start(actB[:, ki, :], actd[KQ + ki][:, :])
                for q, a_sb in ((0, actA), (1, actB)):
                    for m in range(MO):
                        wlt = wlp.tile([P, KQ, P], dt_in, tag="wl")
                        nc.sync.dma_start(wlt[:], wl[:, m, q * KQ:(q + 1) * KQ])
                        # Whole-C output tile: one y DMA per (q, m) with
                        # 4.3KB/partition descriptors instead of 5x 860B.
                        o_sb = op.tile([P, C], dt_in, tag="o")
                        for (c0, tb) in chunks:
                            p2 = ps2.tile([P, TB], f32, tag="ps2")
                            for k in range(KQ):
                                nc.tensor.matmul(
                                    p2[:, :tb], lhsT=wlt[:, k, :],
                                    rhs=a_sb[:, k, c0:c0 + tb],
                                    start=(k == 0), stop=(k == KQ - 1),
                                )
                            nc.vector.tensor_copy(out=o_sb[:, c0:c0 + tb],
                                                  in_=p2[:, :tb])
                        nc.sync.dma_start(yT[q][m][:, :], o_sb[:, :])
    nc.compile()
    return nc


MOE_STRUCT = os.environ.get("MOE_STRUCT", "v2")


def _get_nc(C, F, H, dtype_name):
    key = (C, F, H, dtype_name, TB, MOE_STRUCT)
    if key not in _NEFF_CACHE:
        build = {"dram_act": _build_nc_dram_act, "v2": _build_nc_v2}.get(
            MOE_STRUCT, _build_nc)
        _NEFF_CACHE[key] = build(C, F, H, dtype_name)
    return _NEFF_CACHE[key]


def _mm_dt(mybir, dtype_name):
    return {
        "f32r": mybir.dt.float32r,
        "bf16": mybir.dt.bfloat16,
        "f16": mybir.dt.float16,
    }[dtype_name]


def _np_in_dtype():
    if MOE_DTYPE == "f32r":
        return np.float32
    if MOE_DTYPE == "f16":
        return np.float16
    import ml_dtypes

    return ml_dtypes.bfloat16


def run(x, w_router, w_gating, w_linear, per_expert_scale, router_scale, top_k,
        trace=False):
    from concourse.bass_utils import run_bass_kernel_spmd

    x = np.asarray(x)
    w_router = np.asarray(w_router)
    w_gating = np.asarray(w_gating)
    w_linear = np.asarray(w_linear)
    per_expert_scale = np.asarray(per_expert_scale)
    router_scale = np.asarray(router_scale)
    k = int(top_k)

    G, S, F = x.shape
    E = w_router.shape[1]
    H = w_linear.shape[1]
    B = G * S
    assert E == 8, "expert-parallel mapping assumes 8 experts on 8 cores"
    KF, KH, MO = F // P, H // P, F // P

    choices, combine = _route(x, w_router, router_scale, k)
    wcopy = combine * per_expert_scale.astype(np.float32)[choices]

    cf = choices.reshape(-1)
    tok_of_copy = np.repeat(np.arange(B), k)
    idx_per_e = [np.nonzero(cf == e)[0] for e in range(E)]
    counts = np.array([len(ix) for ix in idx_per_e])
    C = max(512, int(-(-counts.max() // 32)) * 32)

    nc = _get_nc(C, F, H, MOE_DTYPE)
    dt_in = _np_in_dtype()

    xf = x.reshape(B, F)
    if MOE_STRUCT == "v2":
        chunks, TBP, NBLK = _even_chunks_uniform(C, TB)
    in_maps = []
    toks_per_e = []
    for e in range(E):
        toks = tok_of_copy[idx_per_e[e]]
        toks_per_e.append(toks)
        n_e = len(toks)
        if MOE_STRUCT == "v2":
            # xT [P, NBLK, KF, TBP] chunk-major: xT[p, c, ko, j] =
            # x[toks[chunk_c_start + j], ko*P + p]
            xc = np.zeros((C, KF, P), dtype=dt_in)
            xc[:n_e] = xf[toks].astype(dt_in).reshape(n_e, KF, P)
            xT = np.zeros((P, NBLK, KF, TBP), dtype=dt_in)
            for c, (c0, tb) in enumerate(chunks):
                xT[:, c, :, :tb] = xc[c0:c0 + tb].transpose(2, 1, 0)
        else:
            # xT [P, KF, C]: xT[p, ko, c] = x[toks[c], ko*P + p]
            xT = np.zeros((P, KF, C), dtype=dt_in)
            xT[:, :, :n_e] = (
                xf[toks].astype(dt_in).reshape(n_e, KF, P).transpose(2, 1, 0)
            )
        # wg [P, MG, KF, P]: m=2i+c -> gate (c=0) / lin (c=1) rows 128i..128i+127
        wgq = w_gating[e].reshape(2, KH, P, KF, P)        # (c, i, col, ko, p)
        wgt = np.ascontiguousarray(
            wgq.transpose(4, 1, 0, 3, 2).reshape(P, 2 * KH, KF, P)
        ).astype(dt_in)
        # wl [P, MO, KH, P]: wl[p, m, kh, col] = w_linear[e][kh*P+p, m*P+col]
        wlq = w_linear[e].reshape(KH, P, MO, P)           # (kh, p, m, col)
        wlt = np.ascontiguousarray(wlq.transpose(1, 2, 0, 3)).astype(dt_in)
        in_maps.append({"xT": xT, "wg": wgt, "wl": wlt})

    res = run_bass_kernel_spmd(
        nc, in_maps, core_ids=list(range(E)), trace=trace,
        trace_cores=list(range(E)) if trace else None,
    )

    out = np.zeros((B, F), dtype=np.float32)
    for e in range(E):
        toks = toks_per_e[e]
        n_e = len(toks)
        if n_e == 0:
            continue
        yT = res.results[e]["yT"]                         # [MO, P, C] or [NQ, MO, P, C]
        if yT.ndim == 4:
            yT = yT.sum(axis=0, dtype=np.float32)
        y = yT.transpose(2, 0, 1).reshape(C, F)[:n_e]
        w = wcopy.reshape(-1)[idx_per_e[e]][:, None]
        out[toks] += w * y
    return out.reshape(G, S, F), res


def kernel(**inputs) -> np.ndarray:
    out, _ = run(**inputs)
    return out

